# revision 71
# baseline (speedup 1.0000x reference)
"""Multi-head attention (B=2, S=2048, H=2048, NH=16, HD=128) on 8 trn2 cores.

Sharding: core i -> (batch b = i // 4, head-group g = i % 4, 4 heads each).
Each core computes q/k/v projections for its 4 heads, causal-masked
attention, and a partial output projection against its 512-row slice of
Wo.  The host sums the 4 partial outputs per batch.

Layout strategy (everything K-major so no on-chip transposes are needed):
  - host ships x^T (per batch) in bf16; projections compute q^T/k^T
    [d, t] via lhsT=W, rhs=x^T, and v [T, d] via lhsT=x^T, rhs=Wv.
  - scores^T [T, t] = (k^T).T @ q^T; exp on ACT (no max subtraction --
    scores are O(6) here, exp is safe in fp32); runtime mask applied
    multiplicatively AFTER exp (so softmax denominators stay exact).
  - causal staircase blocks: the score / od matmuls and the exp are
    restricted to the visible query range [delta, TBLK); the invisible
    prefix of the e tile is zeroed (DVE) so the denominator tree can
    keep full-width adds.
  - softmax denominators: e tiles accumulate on DVE into a bf16 esum
    tree, reduced across partitions with one ones-matmul per (head,
    block); o^T [d, t] = v.T @ e accumulates in PSUM; the reciprocal
    of the denominator row is partition-broadcast on the (otherwise
    idle) GpSimd engine and multiplied in on DVE, with the multiply
    deferred by one head so its wait never head-of-line blocks the
    DVE queue.
  - final: out[t, m] = (o^T).T @ Wo_rows, accumulated over the 4 heads,
    stored as bf16 partials (host upcasts and sums).

The mask is inspected on the host and the kernel is specialized per
128x512 block: skip (all False), full (all True), affine (causal
staircase), or partial (loads the mask tile and multiplies).

Startup: tau0's q/k projections are emitted contraction-chunk-outer
with all 8 PSUM banks open so each weight tile is consumed for 8 head
matmuls the moment it lands; the two hardware DMA queues are ordered
to match (wq/xt0b on scalar, xt0a/wk interleaved on sync).

Emission is software-pipelined: in query-block tau's slot we emit its
attention heads with the projections of tau+1 and the output-projection
rows of earlier taus as PE filler, so the PE always has independent
matmul work while ACT grinds through the exps.  x blocks prefetch one
tau ahead (split over both queues).

The last query block's output rows accumulate heads 0-2 early (stored
directly as coalesced full rows of `out`), while head 3's term is
projected UNNORMALIZED -- ungated by the final reciprocal chain -- and
the per-query reciprocal, transposed into per-partition columns by
tiny K=1 matmuls, is folded into the PSUM->SBUF copies (DVE
tensor_scalar_mul / ACT Copy-with-scale).  That scaled term ships as a
separate output (out2) and the host adds it into the final rows.
"""

import math

import numpy as np
import ml_dtypes

B, S, H, NH, HD = 2, 2048, 2048, 16, 128
N_CORES = 8
GROUPS = 4                # head-groups (cores per batch)
HPC = NH // GROUPS        # heads per core = 4
DPC = HPC * HD            # head dims per core = 512
TBLK = 512                # query-block width (matmul moving dim)
KBLK = 128                # key-block width (matmul contraction dim)
NT = S // TBLK            # 4 query blocks
NK = S // KBLK            # 16 key blocks
HKT = H // 128            # 16 contraction tiles over hidden dim
HKC = 4                   # contraction chunks per DMA (so loads pipeline)

_BF16 = ml_dtypes.bfloat16

_kernel_cache = {}


MODE_FULL, MODE_AFFINE, MODE_LOADMASK = 0, 1, 2


def _runs(blocks):
    """Group the load-mask blocks of one query block into contiguous Tb
    runs so each run loads with a single DMA."""
    runs = []
    for Tb, mode in blocks:
        if mode != MODE_LOADMASK:
            continue
        if runs and runs[-1][-1] == Tb - 1 and len(runs[-1]) < 4:
            runs[-1].append(Tb)
        else:
            runs.append([Tb])
    return runs


def _build(pattern):
    """Compile the SPMD program for a given mask block pattern.

    pattern: tuple over query-block tau of tuples of (Tb, mode) pairs,
    ascending in Tb, listing key blocks that have any visible entry.
    """
    import concourse.bass as bass  # noqa: F401
    import concourse.tile as tile
    from concourse import bacc, mybir

    fp32 = mybir.dt.float32
    bf16 = mybir.dt.bfloat16
    Exp = mybir.ActivationFunctionType.Exp
    inv_sqrt_hd = 1.0 / math.sqrt(HD)

    all_runs = [_runs(blocks) for blocks in pattern]
    max_run_len = max((len(r) for runs in all_runs for r in runs), default=1)
    max_runs = max((len(runs) for runs in all_runs), default=1)

    nc = bacc.Bacc("TRN2", target_bir_lowering=False, debug=False,
                   num_devices=N_CORES)
    xT = nc.dram_tensor("xT", [H, S], bf16, kind="ExternalInput")
    wq = nc.dram_tensor("wq", [H, DPC], bf16, kind="ExternalInput")
    wk = nc.dram_tensor("wk", [H, DPC], bf16, kind="ExternalInput")
    wv = nc.dram_tensor("wv", [H, DPC], bf16, kind="ExternalInput")
    wo = nc.dram_tensor("wo", [DPC, H], bf16, kind="ExternalInput")
    maskT = nc.dram_tensor("maskT", [S, S], bf16, kind="ExternalInput")
    out = nc.dram_tensor("out", [S, H], bf16, kind="ExternalOutput")
    # the final query block's head-3 contribution ships separately (scaled
    # by the reciprocal on-chip, summed into `out` rows S-TBLK..S on host)
    out2 = nc.dram_tensor("out2", [TBLK, H], bf16, kind="ExternalOutput")

    n_chunks = HKT // HKC  # 4

    with tile.TileContext(nc) as tc:
        with (
            tc.tile_pool(name="persist", bufs=1) as persist,
            tc.tile_pool(name="xt", bufs=8) as xt_pool,
            tc.tile_pool(name="masks", bufs=max(2 * max_runs, 2)) as mask_pool,
            tc.tile_pool(name="e", bufs=9) as e_pool,
            tc.tile_pool(name="outsb", bufs=4) as out_pool,
            tc.tile_pool(name="finsb", bufs=16) as fin_pool,
            tc.tile_pool(name="esum", bufs=6) as esum_pool,
            tc.tile_pool(name="rp", bufs=2) as r_pool,
            tc.tile_pool(name="Rp", bufs=2) as R_pool,
            tc.tile_pool(name="ps_work", bufs=3, space="PSUM") as ps_work,
            tc.tile_pool(name="ps_score", bufs=3, space="PSUM") as ps_score,
            tc.tile_pool(name="ps_acc", bufs=2, space="PSUM") as ps_acc,
        ):
            # --- persistent SBUF tensors -------------------------------
            # DMA queue discipline: first-needed chunks at the head of
            # both hardware DMA queues (sync, scalar); never tensor (its
            # sequencer must stay on the matmul stream), never gpsimd
            # (software DGE).
            # wq/wk load at single-tile granularity so arrival granularity
            # matches the chunk-interleaved startup consumption (one wq +
            # one wk tile feeds 8 matmuls = ~1.7us of PE work).
            WCHS = {"wq": 1, "wk": 1, "wv": 2}
            w_sbs = {n: [None] * (HKT // w) for n, w in WCHS.items()}
            xt0_tiles = [None] * n_chunks

            def _load_w(name, dram, c, eng):
                W = WCHS[name]
                t = persist.tile([128, W, DPC], bf16, tag=f"{name}{c}")
                eng.dma_start(
                    t[:],
                    dram.ap()[c * W * 128:(c + 1) * W * 128, :]
                    .rearrange("(k p) d -> p k d", p=128))
                w_sbs[name][c] = t

            def _load_xt0(c, eng):
                t = xt_pool.tile([128, HKC, TBLK], bf16, tag="xt")
                eng.dma_start(
                    t[:],
                    xT.ap()[c * HKC * 128:(c + 1) * HKC * 128, 0:TBLK]
                    .rearrange("(k p) t -> p k t", p=128))
                xt0_tiles[c] = t

            # Only sync (SP) and scalar (ACT) have hardware DMA queues;
            # gpsimd DMA is software DGE (Pool-generated descriptors,
            # several us per transfer) and must not gate startup.
            # Queue programs are ordered to match the chunk-interleaved
            # startup consumption (hk-outer over q&k heads):
            #   scalar: wq0, xt0b, wq1-15, wv, wo
            #   sync:   xt0a, wk0-1, xt0c1, wk2-5, xt0c2, wk6-9, xt0c3,
            #           wk10-15
            _load_w("wq", wq, 0, nc.scalar)
            # xt chunk 0 gates the first matmul: split it in halves
            # across both hardware queues so the gate is 256KB, not 512KB
            t0 = xt_pool.tile([128, HKC, TBLK], bf16, tag="xt")
            nc.sync.dma_start(
                t0[:, 0:2, :],
                xT.ap()[0:256, 0:TBLK].rearrange("(k p) t -> p k t", p=128))
            nc.scalar.dma_start(
                t0[:, 2:4, :],
                xT.ap()[256:512, 0:TBLK].rearrange("(k p) t -> p k t", p=128))
            xt0_tiles[0] = t0
            _load_w("wk", wk, 0, nc.sync)
            _load_w("wk", wk, 1, nc.sync)
            _load_xt0(1, nc.sync)
            for c in (2, 3, 4, 5):
                _load_w("wk", wk, c, nc.sync)
            _load_xt0(2, nc.sync)
            for c in (6, 7, 8, 9):
                _load_w("wk", wk, c, nc.sync)
            _load_xt0(3, nc.sync)
            for c in range(10, HKT):
                _load_w("wk", wk, c, nc.sync)
            for c in range(1, HKT):
                _load_w("wq", wq, c, nc.scalar)
            for c in range(HKT // WCHS["wv"]):
                _load_w("wv", wv, c, nc.scalar)
            wo_sb = persist.tile([128, HPC, H], bf16, tag="wo")
            nc.scalar.dma_start(
                wo_sb[:], wo.ap().rearrange("(c p) m -> p c m", p=128))

            qT_sb = persist.tile([128, HPC, S], bf16, tag="qT")
            kT_sb = persist.tile([128, HPC, S], bf16, tag="kT")
            v_sb = persist.tile([128, NK, DPC], bf16, tag="v")
            oT_sb = persist.tile([128, HPC, S], bf16, tag="oT")

            ones_bf_sb = persist.tile([128, 1], bf16, tag="ones_bf")
            nc.vector.memset(ones_bf_sb[:], 1.0)
            ones_row_sb = persist.tile([1, 128], bf16, tag="ones_row")
            nc.vector.memset(ones_row_sb[:], 1.0)
            ones_f32_sb = persist.tile([1, 1], fp32, tag="ones_f32")
            nc.vector.memset(ones_f32_sb[:], 1.0)
            # final head's output tile, kept UNNORMALIZED (the per-query
            # reciprocal is folded into the output blocks afterwards)
            o3_sb = persist.tile([128, TBLK], bf16, tag="o3")
            rcol_sb = persist.tile([128, TBLK // 128], fp32, tag="rcol")

            def w_chunk(name, hk):
                W = WCHS[name]
                return w_sbs[name][hk // W][:, hk % W, :]

            xts = {0: xt0_tiles}

            def emit_xt_load(tau):
                if tau in xts:
                    return
                tsl = slice(tau * TBLK, (tau + 1) * TBLK)
                xts[tau] = []
                for c in range(n_chunks):
                    t = xt_pool.tile([128, HKC, TBLK], bf16, tag="xt")
                    # split across both hardware queues so the whole 2MB
                    # block lands in ~3us instead of ~6us -- the next
                    # tau's projection fillers start consuming it almost
                    # immediately.
                    eng = nc.sync if c % 2 == 0 else nc.scalar
                    eng.dma_start(
                        t[:],
                        xT.ap()[c * HKC * 128:(c + 1) * HKC * 128, tsl]
                        .rearrange("(k p) t -> p k t", p=128))
                    xts[tau].append(t)

            def xt_chunk(tau, hk):
                return xts[tau][hk // HKC][:, hk % HKC, :]

            def emit_qk_proj(tau, wname, h):
                tsl = slice(tau * TBLK, (tau + 1) * TBLK)
                dst = qT_sb if wname == "wq" else kT_sb
                ps = ps_work.tile([128, TBLK], fp32, tag="ps")
                for hk in range(HKT):
                    nc.tensor.matmul(
                        ps[:],
                        lhsT=w_chunk(wname, hk)[:, h * HD:(h + 1) * HD],
                        rhs=xt_chunk(tau, hk),
                        start=(hk == 0), stop=(hk == HKT - 1))
                nc.vector.tensor_copy(out=dst[:, h, tsl], in_=ps[:])

            def emit_v_proj(tau, tb_local):
                ps = ps_work.tile([128, TBLK], fp32, tag="ps")
                for hk in range(HKT):
                    nc.tensor.matmul(
                        ps[:],
                        lhsT=xt_chunk(tau, hk)[:, tb_local * KBLK:(tb_local + 1) * KBLK],
                        rhs=w_chunk("wv", hk),
                        start=(hk == 0), stop=(hk == HKT - 1))
                nc.vector.tensor_copy(
                    out=v_sb[:, tau * (TBLK // KBLK) + tb_local, :], in_=ps[:])

            mask_tiles = {}

            def emit_mask_loads(tau):
                tsl = slice(tau * TBLK, (tau + 1) * TBLK)
                for run in all_runs[tau]:
                    mt = mask_pool.tile([128, max_run_len, TBLK], bf16,
                                        tag="mask")
                    nc.sync.dma_start(
                        mt[:, :len(run), :],
                        maskT.ap()[run[0] * KBLK:(run[-1] + 1) * KBLK, tsl]
                        .rearrange("(k p) t -> p k t", p=128))
                    for j, Tb in enumerate(run):
                        mask_tiles[(tau, Tb)] = mt[:, j, :]

            def emit_attention_head(tau, h, chunk=3):
                """Generator; yields ('chunk',), ('pre_dn',), ('post_dn',)
                at points where the caller may emit PE filler."""
                tsl = slice(tau * TBLK, (tau + 1) * TBLK)
                blocks = pattern[tau]
                od = ps_acc.tile([128, TBLK], fp32, tag="od")
                tree = []  # (level, tile) stack for streaming bf16 sum tree

                def emit_score(i, Tb, mode):
                    # causal-staircase blocks only see queries >= delta;
                    # restrict the matmuls/exp to that range (the first
                    # block of the row is always full width, so the PSUM
                    # zero-region accumulate rules are satisfied).
                    w0 = 0
                    if mode == MODE_AFFINE:
                        delta = Tb * KBLK - tau * TBLK
                        if 0 < delta < TBLK and i > 0:
                            w0 = delta
                    sp = ps_score.tile([128, TBLK], fp32, tag="sc")
                    nc.tensor.matmul(
                        sp[:, w0:],
                        lhsT=kT_sb[:, h, Tb * KBLK:(Tb + 1) * KBLK],
                        rhs=qT_sb[:, h, tau * TBLK + w0:(tau + 1) * TBLK],
                        start=True, stop=True)
                    e = e_pool.tile([128, TBLK], bf16, tag="e")
                    if w0:
                        # zero prefix keeps the denominator tree adds
                        # full-width.  DVE, not gpsimd: gpsimd carries
                        # the reciprocal partition-broadcasts and the
                        # affine_selects, and a queued memset behind a
                        # ~1us broadcast stalls this block's od matmul.
                        nc.vector.memset(e[:, :w0], 0.0)
                    nc.scalar.activation(out=e[:, w0:], in_=sp[:, w0:],
                                         func=Exp, scale=inv_sqrt_hd)
                    if mode == MODE_AFFINE:
                        # zero entries where t_rel - T_rel < delta
                        # (value = -(delta-w0) + t'_rel - T_rel, keep >= 0)
                        delta = Tb * KBLK - tau * TBLK
                        nc.gpsimd.affine_select(
                            out=e[:, w0:], in_=e[:, w0:],
                            compare_op=mybir.AluOpType.is_ge,
                            fill=0.0, base=-(delta - w0),
                            pattern=[[1, TBLK - w0]], channel_multiplier=-1)
                    elif mode == MODE_LOADMASK:
                        nc.vector.tensor_mul(e[:], e[:], mask_tiles[(tau, Tb)])
                    return e, w0

                def emit_od(i, Tb, e, w0):
                    cur, lvl = e, 0
                    while tree and tree[-1][0] == lvl:
                        _, prev = tree.pop()
                        acc = esum_pool.tile([128, TBLK], bf16, tag="esum")
                        nc.vector.tensor_add(acc[:], prev[:], cur[:])
                        cur, lvl = acc, lvl + 1
                    tree.append((lvl, cur))
                    nc.tensor.matmul(
                        od[:, w0:],
                        lhsT=v_sb[:, Tb, h * HD:(h + 1) * HD],
                        rhs=e[:, w0:],
                        start=(i == 0), stop=(i == len(blocks) - 1))

                # software-pipelined by one block: block i+1's score
                # matmul sits between score_i and od_i in the PE queue,
                # so od_i's wait on exp_i drains behind independent work
                # instead of an exposed pipeline refill (~100ns/block).
                pend = None
                for i, (Tb, mode) in enumerate(blocks):
                    if i and i % chunk == 0:
                        yield "chunk"
                    e, w0 = emit_score(i, Tb, mode)
                    if pend is not None:
                        emit_od(*pend)
                    pend = (i, Tb, e, w0)
                emit_od(*pend)
                yield "pre_dn"
                while len(tree) > 1:
                    _, a = tree.pop()
                    _, b2 = tree.pop()
                    acc = esum_pool.tile([128, TBLK], bf16, tag="esum")
                    nc.vector.tensor_add(acc[:], a[:], b2[:])
                    tree.append((99, acc))
                esum = tree.pop()[1]
                dn = ps_score.tile([1, TBLK], fp32, tag="sc")
                nc.tensor.matmul(dn[:], lhsT=ones_bf_sb[:], rhs=esum[:],
                                 start=True, stop=True)
                yield "post_dn"
                if tau == NT - 1 and h == HPC - 1:
                    # final head: leave the output UNNORMALIZED (copied to
                    # SBUF on ACT, ungated by the reciprocal) and instead
                    # transpose the reciprocal into per-partition columns
                    # with tiny K=1 matmuls; the final output blocks fold
                    # the scale in with one fused scalar_tensor_tensor
                    # each, so the 16 tail matmuls never wait on the
                    # reciprocal chain.
                    # o3 copied in 128-col pieces so the first tail
                    # matmul (needs only its own slice) starts ~0.5us
                    # earlier than a monolithic 512-col copy would allow
                    for j in range(TBLK // 128):
                        nc.scalar.activation(
                            out=o3_sb[:, j * 128:(j + 1) * 128],
                            in_=od[:, j * 128:(j + 1) * 128], func=Copy)
                    r = r_pool.tile([1, TBLK], fp32, tag="r")
                    nc.vector.reciprocal_approx_fast(out=r[:], in_=dn[:])
                    rcp = ps_score.tile([128, TBLK], fp32, tag="sc")
                    for j in range(TBLK // 128):
                        nc.tensor.matmul(
                            rcp[:, j:j + 1],
                            lhsT=r[:, j * 128:(j + 1) * 128],
                            rhs=ones_f32_sb[:],
                            start=True, stop=True)
                    nc.vector.tensor_copy(out=rcol_sb[:],
                                          in_=rcp[:, 0:TBLK // 128])
                else:
                    r = r_pool.tile([1, TBLK], fp32, tag="r")
                    nc.vector.reciprocal_approx_fast(out=r[:], in_=dn[:])
                    r_bf = r_pool.tile([1, TBLK], bf16, tag="rb", bufs=1)
                    nc.vector.tensor_copy(out=r_bf[:], in_=r[:])
                    # partition-broadcast of the reciprocal on the (idle)
                    # GpSimd engine -- no DRAM bounce, no PE matmul; bf16
                    # halves the broadcast and the mul's second operand
                    R = R_pool.tile([128, TBLK], bf16, tag="R")
                    nc.gpsimd.partition_broadcast(R[:], r_bf[:])

                    # the normalize mul waits for the broadcast; emit it
                    # from the NEXT head so it doesn't head-of-line block
                    # the DVE queue (esum adds) behind that wait.
                    def _norm(od=od, R=R, h=h, tsl=tsl):
                        nc.vector.tensor_mul(oT_sb[:, h, tsl], od[:], R[:])
                    yield ("norm", _norm)

            Copy = mybir.ActivationFunctionType.Copy

            def emit_out_block(tt, mb, store_eng=None, copy_eng=None):
                # one 128x512 block of the final projection
                ps = ps_work.tile([128, TBLK], fp32, tag="ps")
                for h in range(HPC):
                    nc.tensor.matmul(
                        ps[:],
                        lhsT=oT_sb[:, h, tt * 128:(tt + 1) * 128],
                        rhs=wo_sb[:, h, mb * TBLK:(mb + 1) * TBLK],
                        start=(h == 0), stop=(h == HPC - 1))
                osb = out_pool.tile([128, TBLK], bf16, tag="osb")
                if copy_eng is nc.scalar:
                    nc.scalar.activation(out=osb[:], in_=ps[:], func=Copy)
                else:
                    (copy_eng or nc.vector).tensor_copy(out=osb[:], in_=ps[:])
                (store_eng or nc.sync).dma_start(
                    out.ap()[tt * 128:(tt + 1) * 128,
                             mb * TBLK:(mb + 1) * TBLK],
                    osb[:])

            # ---- emission schedule -----------------------------------
            # Output rows ride as PE filler: rows 0-3 (query block 0) in
            # slot 2, rows 4-11 in slot 3 (tau=3's attention is the most
            # ACT-bound stretch and otherwise starves the PE), rows 12-15
            # after the final head's chain, with stores on the scalar
            # queue so the final reciprocal bounce never queues behind
            # them on sync.
            rows_per_tau = TBLK // 128
            row_slot = {2: [0, 1, 2, 3], 3: [4, 5, 6, 7, 8, 9, 10, 11]}



            # projections for tau=0 run standalone (startup).  q and k
            # are emitted hk-outer with 8 simultaneously-open PSUM groups
            # so each weight tile is consumed for all 8 head-matmuls the
            # moment it lands -- the PE then tracks the two DMA streams
            # (wq on scalar, wk on sync) instead of stalling head-serial.
            qps = [ps_work.tile([128, TBLK], fp32, tag="ps", name=f"qp{i}")
                   for i in range(3)]
            qps.append(ps_acc.tile([128, TBLK], fp32, tag="od", name="qp3"))
            kps = [ps_score.tile([128, TBLK], fp32, tag="sc", name=f"kp{i}")
                   for i in range(3)]
            kps.append(ps_acc.tile([128, TBLK], fp32, tag="od", name="kp3"))
            for hk in range(HKT):
                for h in range(HPC):
                    nc.tensor.matmul(
                        qps[h][:],
                        lhsT=w_chunk("wq", hk)[:, h * HD:(h + 1) * HD],
                        rhs=xt_chunk(0, hk),
                        start=(hk == 0), stop=(hk == HKT - 1))
                for h in range(HPC):
                    nc.tensor.matmul(
                        kps[h][:],
                        lhsT=w_chunk("wk", hk)[:, h * HD:(h + 1) * HD],
                        rhs=xt_chunk(0, hk),
                        start=(hk == 0), stop=(hk == HKT - 1))
            for h in range(HPC):
                if h % 2:
                    nc.scalar.activation(out=qT_sb[:, h, 0:TBLK],
                                         in_=qps[h][:], func=Copy)
                    nc.vector.tensor_copy(out=kT_sb[:, h, 0:TBLK],
                                          in_=kps[h][:])
                else:
                    nc.vector.tensor_copy(out=qT_sb[:, h, 0:TBLK],
                                          in_=qps[h][:])
                    nc.scalar.activation(out=kT_sb[:, h, 0:TBLK],
                                         in_=kps[h][:], func=Copy)
            # tau1's x lands well before tau0's attention fillers start
            # consuming it (the tau-loop emission point is only ~0-6us
            # ahead of first use for tau==0)
            emit_xt_load(1)
            emit_mask_loads(1)
            masks_loaded = {1}

            for tb in range(rows_per_tau):
                emit_v_proj(0, tb)

            emit_mask_loads(0)

            deferred_norm = None
            for tau in range(NT):
                fillers = []
                if tau + 1 < NT:
                    emit_xt_load(tau + 1)
                    if tau + 1 not in masks_loaded:
                        emit_mask_loads(tau + 1)
                        masks_loaded.add(tau + 1)
                    fillers += [lambda h=h, t=tau + 1:
                                emit_qk_proj(t, "wq", h)
                                for h in range(HPC)]
                    fillers += [lambda h=h, t=tau + 1:
                                emit_qk_proj(t, "wk", h)
                                for h in range(HPC)]
                    fillers += [lambda tb=tb, t=tau + 1:
                                emit_v_proj(t, tb)
                                for tb in range(rows_per_tau)]
                for tt in row_slot.get(tau, []):
                    fillers += [lambda tt=tt, mb=mb: emit_out_block(tt, mb)
                                for mb in range(H // TBLK)]
                fill_iter = iter(fillers)

                def fill(n=1):
                    for _ in range(n):
                        f = next(fill_iter, None)
                        if f is None:
                            return
                        f()

                for h in range(HPC):
                    for ev in emit_attention_head(tau, h):
                        if ev in ("chunk", "pre_dn"):
                            if deferred_norm is not None:
                                deferred_norm()
                                deferred_norm = None
                            fill(1)
                        elif isinstance(ev, tuple) and ev[0] == "norm":
                            deferred_norm = ev[1]
                    fill(1)
                for f in fill_iter:
                    f()

            # Final query block's output rows, fully interleaved: per
            # (tt, mb) block, the heads-0-2 partial (3 matmuls + cast
            # into a coalesced row tile, stored to `out`) is followed
            # immediately by the head-3 term against the UNNORMALIZED
            # o3_sb (1 matmul; the per-query reciprocal is folded into
            # the PSUM->SBUF copy and ships via out2).  Interleaving
            # keeps both copy engines draining tail copies from the
            # start instead of queueing them behind all 16 partial
            # casts, so the PSUM rings never pace the matmuls.  The two
            # per-block copies go to opposite engines.
            tail_pools = [(ps_acc, "od"), (ps_score, "sc")]
            j = 0
            for tt in range((NT - 1) * rows_per_tau, NT * rows_per_tau):
                ttl = tt - (NT - 1) * rows_per_tau
                orow = fin_pool.tile([128, H], bf16, tag="fin", bufs=4,
                                     name="orow")
                for mb in range(H // TBLK):
                    ps = ps_work.tile([128, TBLK], fp32, tag="ps")
                    for h in range(HPC - 1):
                        nc.tensor.matmul(
                            ps[:],
                            lhsT=oT_sb[:, h, tt * 128:(tt + 1) * 128],
                            rhs=wo_sb[:, h, mb * TBLK:(mb + 1) * TBLK],
                            start=(h == 0), stop=(h == HPC - 2))
                    osb = orow[:, mb * TBLK:(mb + 1) * TBLK]
                    if j % 2:
                        nc.scalar.activation(out=osb, in_=ps[:], func=Copy)
                    else:
                        nc.vector.tensor_copy(out=osb, in_=ps[:])

                    pool_, tag_ = tail_pools[j % 2]
                    ps2 = pool_.tile([128, TBLK], fp32, tag=tag_, name="ps2")
                    nc.tensor.matmul(
                        ps2[:],
                        lhsT=o3_sb[:, ttl * 128:(ttl + 1) * 128],
                        rhs=wo_sb[:, HPC - 1, mb * TBLK:(mb + 1) * TBLK],
                        start=True, stop=True)
                    if j % 2:
                        o2sb = fin_pool.tile([128, TBLK], bf16, tag="fin2v",
                                             bufs=2)
                        nc.vector.tensor_scalar_mul(
                            out=o2sb[:], in0=ps2[:],
                            scalar1=rcol_sb[:, ttl:ttl + 1])
                    else:
                        o2sb = fin_pool.tile([128, TBLK], bf16, tag="fin2s",
                                             bufs=2)
                        nc.scalar.activation(out=o2sb[:], in_=ps2[:],
                                             func=Copy,
                                             scale=rcol_sb[:, ttl:ttl + 1])
                    (nc.scalar if j % 2 else nc.sync).dma_start(
                        out2.ap()[ttl * 128:(ttl + 1) * 128,
                                  mb * TBLK:(mb + 1) * TBLK],
                        o2sb[:])
                    j += 1
                nc.sync.dma_start(
                    out.ap()[tt * 128:(tt + 1) * 128, :], orow[:])

    nc.compile()
    return nc


def _classify(mask):
    """Per 128x512 block of mask^T: skip / full / affine / partial,
    unioned over batches.  Returns the pattern tuple, or None if some
    row is fully masked (degenerate -- reference gives uniform weights
    there)."""
    if not mask.any(axis=2).all():
        return None
    tr = np.arange(TBLK)[:, None]
    Tr = np.arange(KBLK)[None, :]
    pattern = []
    for tau in range(NT):
        blocks = []
        for Tb in range(NK):
            # block of mask^T[Tb*128:(Tb+1)*128, tau*512:(tau+1)*512]
            # == mask[:, tau*512:(tau+1)*512, Tb*128:(Tb+1)*128]
            blk = mask[:, tau * TBLK:(tau + 1) * TBLK,
                       Tb * KBLK:(Tb + 1) * KBLK]
            if not blk.any():
                continue
            if blk.all():
                blocks.append((Tb, MODE_FULL))
                continue
            # causal staircase? mask[t, T] = (t >= T), i.e.
            # tau*TBLK + tr >= Tb*KBLK + Tr
            stair = (tau * TBLK + tr) >= (Tb * KBLK + Tr)
            if all((blk[b] == stair).all() for b in range(blk.shape[0])):
                blocks.append((Tb, MODE_AFFINE))
            else:
                blocks.append((Tb, MODE_LOADMASK))
        pattern.append(tuple(blocks))
    return tuple(pattern)


def _reference_fallback(x, mask, Wq, Wk, Wv, Wo):
    out = np.empty((B, S, H), np.float32)
    for b in range(B):
        q = (x[b] @ Wq).reshape(S, NH, HD).transpose(1, 0, 2)
        k = (x[b] @ Wk).reshape(S, NH, HD).transpose(1, 0, 2)
        v = (x[b] @ Wv).reshape(S, NH, HD).transpose(1, 0, 2)
        s = np.einsum("htd,hTd->htT", q, k) / np.sqrt(HD)
        s = np.where(mask[b][None], s, -1e10)
        s -= s.max(-1, keepdims=True)
        w = np.exp(s)
        w /= w.sum(-1, keepdims=True)
        o = np.einsum("htT,hTd->htd", w, v)
        out[b] = o.transpose(1, 0, 2).reshape(S, NH * HD) @ Wo
    return out


def kernel(x, mask, Wq, Wk, Wv, Wo):
    x = np.asarray(x, np.float32)
    mask = np.asarray(mask).astype(bool)
    Wq = np.asarray(Wq, np.float32)
    Wk = np.asarray(Wk, np.float32)
    Wv = np.asarray(Wv, np.float32)
    Wo = np.asarray(Wo, np.float32)
    assert x.shape == (B, S, H) and mask.shape == (B, S, S)

    pattern = _classify(mask)
    if pattern is None:
        return _reference_fallback(x, mask, Wq, Wk, Wv, Wo)

    if pattern not in _kernel_cache:
        try:
            _kernel_cache[pattern] = _build(pattern)
        except Exception:
            # exotic mask patterns can exceed the SBUF budget (mask-tile
            # pool); stay correct via the host path
            _kernel_cache[pattern] = None
    nc = _kernel_cache[pattern]
    if nc is None:
        return _reference_fallback(x, mask, Wq, Wk, Wv, Wo)

    xT_b = [np.ascontiguousarray(x[b].T).astype(_BF16) for b in range(B)]
    maskT_b = [np.ascontiguousarray(mask[b].T).astype(_BF16) for b in range(B)]
    wq_g = [np.ascontiguousarray(Wq[:, g * DPC:(g + 1) * DPC]).astype(_BF16)
            for g in range(GROUPS)]
    wk_g = [np.ascontiguousarray(Wk[:, g * DPC:(g + 1) * DPC]).astype(_BF16)
            for g in range(GROUPS)]
    wv_g = [np.ascontiguousarray(Wv[:, g * DPC:(g + 1) * DPC]).astype(_BF16)
            for g in range(GROUPS)]
    wo_g = [np.ascontiguousarray(Wo[g * DPC:(g + 1) * DPC, :]).astype(_BF16)
            for g in range(GROUPS)]

    in_maps = []
    for i in range(N_CORES):
        b, g = divmod(i, GROUPS)
        in_maps.append({
            "xT": xT_b[b], "maskT": maskT_b[b],
            "wq": wq_g[g], "wk": wk_g[g], "wv": wv_g[g], "wo": wo_g[g],
        })

    from concourse.bass_utils import run_bass_kernel_spmd
    res = run_bass_kernel_spmd(nc, in_maps, core_ids=list(range(N_CORES)))

    out = np.zeros((B, S, H), np.float32)
    for i in range(N_CORES):
        b = i // GROUPS
        out[b] += res.results[i]["out"].astype(np.float32)
        # final query block's head-3 term ships separately
        out[b, S - TBLK:] += res.results[i]["out2"].astype(np.float32)
    return out



# revision 72
# speedup vs baseline: 1.0161x; 1.0161x over previous
"""Multi-head attention (B=2, S=2048, H=2048, NH=16, HD=128) on 8 trn2 cores.

Sharding: core i -> (batch b = i // 4, head-group g = i % 4, 4 heads each).
Each core computes q/k/v projections for its 4 heads, causal-masked
attention, and a partial output projection against its 512-row slice of
Wo.  The host sums the 4 partial outputs per batch.

Layout strategy (everything K-major so no on-chip transposes are needed):
  - host ships x^T (per batch) in bf16; projections compute q^T/k^T
    [d, t] via lhsT=W, rhs=x^T, and v [T, d] via lhsT=x^T, rhs=Wv.
  - scores^T [T, t] = (k^T).T @ q^T; exp on ACT (no max subtraction --
    scores are O(6) here, exp is safe in fp32); runtime mask applied
    multiplicatively AFTER exp (so softmax denominators stay exact).
  - causal staircase blocks: the score / od matmuls and the exp are
    restricted to the visible query range [delta, TBLK); the invisible
    prefix of the e tile is zeroed (DVE) so the denominator tree can
    keep full-width adds.
  - softmax denominators: e tiles accumulate on DVE into a bf16 esum
    tree, reduced across partitions with one ones-matmul per (head,
    block); o^T [d, t] = v.T @ e accumulates in PSUM; the reciprocal
    of the denominator row is partition-broadcast on the (otherwise
    idle) GpSimd engine and multiplied in on DVE, with the multiply
    deferred by one head so its wait never head-of-line blocks the
    DVE queue.
  - final: out[t, m] = (o^T).T @ Wo_rows, accumulated over the 4 heads,
    stored as bf16 partials (host upcasts and sums).

The mask is inspected on the host and the kernel is specialized per
128x512 block: skip (all False), full (all True), affine (causal
staircase), or partial (loads the mask tile and multiplies).

Startup: tau0's q/k projections are emitted contraction-chunk-outer
with all 8 PSUM banks open so each weight tile is consumed for 8 head
matmuls the moment it lands; the two hardware DMA queues are ordered
to match (wq/xt0b on scalar, xt0a/wk interleaved on sync).

Emission is software-pipelined: in query-block tau's slot we emit its
attention heads with the projections of tau+1 and the output-projection
rows of earlier taus as PE filler, so the PE always has independent
matmul work while ACT grinds through the exps.  x blocks prefetch one
tau ahead (split over both queues).

The last query block's output rows accumulate heads 0-2 early (stored
directly as coalesced full rows of `out`), while head 3's term is
projected UNNORMALIZED -- ungated by the final reciprocal chain -- and
the per-query reciprocal, transposed into per-partition columns by
tiny K=1 matmuls, is folded into the PSUM->SBUF copies (DVE
tensor_scalar_mul / ACT Copy-with-scale).  That scaled term ships as a
separate output (out2) and the host adds it into the final rows.
"""

import math

import numpy as np
import ml_dtypes

B, S, H, NH, HD = 2, 2048, 2048, 16, 128
N_CORES = 8
GROUPS = 4                # head-groups (cores per batch)
HPC = NH // GROUPS        # heads per core = 4
DPC = HPC * HD            # head dims per core = 512
TBLK = 512                # query-block width (matmul moving dim)
KBLK = 128                # key-block width (matmul contraction dim)
NT = S // TBLK            # 4 query blocks
NK = S // KBLK            # 16 key blocks
HKT = H // 128            # 16 contraction tiles over hidden dim
HKC = 4                   # contraction chunks per DMA (so loads pipeline)

_BF16 = ml_dtypes.bfloat16

_kernel_cache = {}


MODE_FULL, MODE_AFFINE, MODE_LOADMASK = 0, 1, 2


def _runs(blocks):
    """Group the load-mask blocks of one query block into contiguous Tb
    runs so each run loads with a single DMA."""
    runs = []
    for Tb, mode in blocks:
        if mode != MODE_LOADMASK:
            continue
        if runs and runs[-1][-1] == Tb - 1 and len(runs[-1]) < 4:
            runs[-1].append(Tb)
        else:
            runs.append([Tb])
    return runs


def _build(pattern):
    """Compile the SPMD program for a given mask block pattern.

    pattern: tuple over query-block tau of tuples of (Tb, mode) pairs,
    ascending in Tb, listing key blocks that have any visible entry.
    """
    import concourse.bass as bass  # noqa: F401
    import concourse.tile as tile
    from concourse import bacc, mybir

    fp32 = mybir.dt.float32
    bf16 = mybir.dt.bfloat16
    Exp = mybir.ActivationFunctionType.Exp
    inv_sqrt_hd = 1.0 / math.sqrt(HD)

    all_runs = [_runs(blocks) for blocks in pattern]
    max_run_len = max((len(r) for runs in all_runs for r in runs), default=1)
    max_runs = max((len(runs) for runs in all_runs), default=1)

    nc = bacc.Bacc("TRN2", target_bir_lowering=False, debug=False,
                   num_devices=N_CORES)
    xT = nc.dram_tensor("xT", [H, S], bf16, kind="ExternalInput")
    wq = nc.dram_tensor("wq", [H, DPC], bf16, kind="ExternalInput")
    wk = nc.dram_tensor("wk", [H, DPC], bf16, kind="ExternalInput")
    wv = nc.dram_tensor("wv", [H, DPC], bf16, kind="ExternalInput")
    wo = nc.dram_tensor("wo", [DPC, H], bf16, kind="ExternalInput")
    maskT = nc.dram_tensor("maskT", [S, S], bf16, kind="ExternalInput")
    out = nc.dram_tensor("out", [S, H], bf16, kind="ExternalOutput")
    # the final query block's head-3 contribution ships separately (scaled
    # by the reciprocal on-chip, summed into `out` rows S-TBLK..S on host)
    out2 = nc.dram_tensor("out2", [TBLK, H], bf16, kind="ExternalOutput")

    n_chunks = HKT // HKC  # 4

    with tile.TileContext(nc) as tc:
        with (
            tc.tile_pool(name="persist", bufs=1) as persist,
            tc.tile_pool(name="xt", bufs=8) as xt_pool,
            tc.tile_pool(name="masks", bufs=max(2 * max_runs, 2)) as mask_pool,
            tc.tile_pool(name="e", bufs=9) as e_pool,
            tc.tile_pool(name="outsb", bufs=4) as out_pool,
            tc.tile_pool(name="finsb", bufs=16) as fin_pool,
            tc.tile_pool(name="esum", bufs=6) as esum_pool,
            tc.tile_pool(name="rp", bufs=2) as r_pool,
            tc.tile_pool(name="Rp", bufs=2) as R_pool,
            tc.tile_pool(name="ps_work", bufs=3, space="PSUM") as ps_work,
            tc.tile_pool(name="ps_score", bufs=3, space="PSUM") as ps_score,
            tc.tile_pool(name="ps_acc", bufs=2, space="PSUM") as ps_acc,
        ):
            # --- persistent SBUF tensors -------------------------------
            # DMA queue discipline: first-needed chunks at the head of
            # both hardware DMA queues (sync, scalar); never tensor (its
            # sequencer must stay on the matmul stream), never gpsimd
            # (software DGE).
            # wq/wk load at single-tile granularity so arrival granularity
            # matches the chunk-interleaved startup consumption (one wq +
            # one wk tile feeds 8 matmuls = ~1.7us of PE work).
            WCHS = {"wq": 1, "wk": 1, "wv": 2}
            w_sbs = {n: [None] * (HKT // w) for n, w in WCHS.items()}
            xt0_tiles = [None] * n_chunks

            def _load_w(name, dram, c, eng):
                W = WCHS[name]
                t = persist.tile([128, W, DPC], bf16, tag=f"{name}{c}")
                eng.dma_start(
                    t[:],
                    dram.ap()[c * W * 128:(c + 1) * W * 128, :]
                    .rearrange("(k p) d -> p k d", p=128))
                w_sbs[name][c] = t

            def _load_xt0(c, eng):
                t = xt_pool.tile([128, HKC, TBLK], bf16, tag="xt")
                eng.dma_start(
                    t[:],
                    xT.ap()[c * HKC * 128:(c + 1) * HKC * 128, 0:TBLK]
                    .rearrange("(k p) t -> p k t", p=128))
                xt0_tiles[c] = t

            # Only sync (SP) and scalar (ACT) have hardware DMA queues;
            # gpsimd DMA is software DGE (Pool-generated descriptors,
            # several us per transfer) and must not gate startup.
            # Queue programs are ordered to match the chunk-interleaved
            # startup consumption (hk-outer over q&k heads):
            #   scalar: wq0, xt0b, wq1-15, wv, wo
            #   sync:   xt0a, wk0-1, xt0c1, wk2-5, xt0c2, wk6-9, xt0c3,
            #           wk10-15
            _load_w("wq", wq, 0, nc.scalar)
            # xt chunk 0 gates the first matmul: split it in halves
            # across both hardware queues so the gate is 256KB, not 512KB
            t0 = xt_pool.tile([128, HKC, TBLK], bf16, tag="xt")
            nc.sync.dma_start(
                t0[:, 0:2, :],
                xT.ap()[0:256, 0:TBLK].rearrange("(k p) t -> p k t", p=128))
            nc.scalar.dma_start(
                t0[:, 2:4, :],
                xT.ap()[256:512, 0:TBLK].rearrange("(k p) t -> p k t", p=128))
            xt0_tiles[0] = t0
            _load_w("wk", wk, 0, nc.sync)
            _load_w("wk", wk, 1, nc.sync)
            _load_xt0(1, nc.sync)
            for c in (2, 3, 4, 5):
                _load_w("wk", wk, c, nc.sync)
            _load_xt0(2, nc.sync)
            for c in (6, 7, 8, 9):
                _load_w("wk", wk, c, nc.sync)
            _load_xt0(3, nc.sync)
            for c in range(10, HKT):
                _load_w("wk", wk, c, nc.sync)
            for c in range(1, HKT):
                _load_w("wq", wq, c, nc.scalar)
            for c in range(HKT // WCHS["wv"]):
                _load_w("wv", wv, c, nc.scalar)
            wo_sb = persist.tile([128, HPC, H], bf16, tag="wo")
            nc.scalar.dma_start(
                wo_sb[:], wo.ap().rearrange("(c p) m -> p c m", p=128))

            qT_sb = persist.tile([128, HPC, S], bf16, tag="qT")
            kT_sb = persist.tile([128, HPC, S], bf16, tag="kT")
            v_sb = persist.tile([128, NK, DPC], bf16, tag="v")
            oT_sb = persist.tile([128, HPC, S], bf16, tag="oT")

            ones_bf_sb = persist.tile([128, 1], bf16, tag="ones_bf")
            nc.vector.memset(ones_bf_sb[:], 1.0)
            ones_row_sb = persist.tile([1, 128], bf16, tag="ones_row")
            nc.vector.memset(ones_row_sb[:], 1.0)
            ones_f32_sb = persist.tile([1, 1], fp32, tag="ones_f32")
            nc.vector.memset(ones_f32_sb[:], 1.0)
            # final head's output tile, kept UNNORMALIZED (the per-query
            # reciprocal is folded into the output blocks afterwards)
            o3_sb = persist.tile([128, TBLK], bf16, tag="o3")
            rcol_sb = persist.tile([128, TBLK // 128], fp32, tag="rcol")

            def w_chunk(name, hk):
                W = WCHS[name]
                return w_sbs[name][hk // W][:, hk % W, :]

            xts = {0: xt0_tiles}

            def emit_xt_load(tau):
                if tau in xts:
                    return
                tsl = slice(tau * TBLK, (tau + 1) * TBLK)
                xts[tau] = []
                for c in range(n_chunks):
                    t = xt_pool.tile([128, HKC, TBLK], bf16, tag="xt")
                    # split across both hardware queues so the whole 2MB
                    # block lands in ~3us instead of ~6us -- the next
                    # tau's projection fillers start consuming it almost
                    # immediately.
                    eng = nc.sync if c % 2 == 0 else nc.scalar
                    eng.dma_start(
                        t[:],
                        xT.ap()[c * HKC * 128:(c + 1) * HKC * 128, tsl]
                        .rearrange("(k p) t -> p k t", p=128))
                    xts[tau].append(t)

            def xt_chunk(tau, hk):
                return xts[tau][hk // HKC][:, hk % HKC, :]

            def emit_qk_proj(tau, wname, h):
                tsl = slice(tau * TBLK, (tau + 1) * TBLK)
                dst = qT_sb if wname == "wq" else kT_sb
                ps = ps_work.tile([128, TBLK], fp32, tag="ps")
                for hk in range(HKT):
                    nc.tensor.matmul(
                        ps[:],
                        lhsT=w_chunk(wname, hk)[:, h * HD:(h + 1) * HD],
                        rhs=xt_chunk(tau, hk),
                        start=(hk == 0), stop=(hk == HKT - 1))
                nc.vector.tensor_copy(out=dst[:, h, tsl], in_=ps[:])

            def emit_v_proj(tau, tb_local):
                ps = ps_work.tile([128, TBLK], fp32, tag="ps")
                for hk in range(HKT):
                    nc.tensor.matmul(
                        ps[:],
                        lhsT=xt_chunk(tau, hk)[:, tb_local * KBLK:(tb_local + 1) * KBLK],
                        rhs=w_chunk("wv", hk),
                        start=(hk == 0), stop=(hk == HKT - 1))
                nc.vector.tensor_copy(
                    out=v_sb[:, tau * (TBLK // KBLK) + tb_local, :], in_=ps[:])

            mask_tiles = {}

            def emit_mask_loads(tau):
                tsl = slice(tau * TBLK, (tau + 1) * TBLK)
                for run in all_runs[tau]:
                    mt = mask_pool.tile([128, max_run_len, TBLK], bf16,
                                        tag="mask")
                    nc.sync.dma_start(
                        mt[:, :len(run), :],
                        maskT.ap()[run[0] * KBLK:(run[-1] + 1) * KBLK, tsl]
                        .rearrange("(k p) t -> p k t", p=128))
                    for j, Tb in enumerate(run):
                        mask_tiles[(tau, Tb)] = mt[:, j, :]

            def emit_attention_head(tau, h, chunk=3):
                """Generator; yields ('chunk',), ('pre_dn',), ('post_dn',)
                at points where the caller may emit PE filler."""
                tsl = slice(tau * TBLK, (tau + 1) * TBLK)
                blocks = pattern[tau]
                od = ps_acc.tile([128, TBLK], fp32, tag="od")
                tree = []  # (level, tile) stack for streaming bf16 sum tree

                def emit_score(i, Tb, mode):
                    # causal-staircase blocks only see queries >= delta;
                    # restrict the matmuls/exp to that range (the first
                    # block of the row is always full width, so the PSUM
                    # zero-region accumulate rules are satisfied).
                    w0 = 0
                    if mode == MODE_AFFINE:
                        delta = Tb * KBLK - tau * TBLK
                        if 0 < delta < TBLK and i > 0:
                            w0 = delta
                    sp = ps_score.tile([128, TBLK], fp32, tag="sc")
                    nc.tensor.matmul(
                        sp[:, w0:],
                        lhsT=kT_sb[:, h, Tb * KBLK:(Tb + 1) * KBLK],
                        rhs=qT_sb[:, h, tau * TBLK + w0:(tau + 1) * TBLK],
                        start=True, stop=True)
                    e = e_pool.tile([128, TBLK], bf16, tag="e")
                    if w0:
                        # zero prefix keeps the denominator tree adds
                        # full-width.  DVE, not gpsimd: gpsimd carries
                        # the reciprocal partition-broadcasts and the
                        # affine_selects, and a queued memset behind a
                        # ~1us broadcast stalls this block's od matmul.
                        nc.vector.memset(e[:, :w0], 0.0)
                    nc.scalar.activation(out=e[:, w0:], in_=sp[:, w0:],
                                         func=Exp, scale=inv_sqrt_hd)
                    if mode == MODE_AFFINE:
                        # zero entries where t_rel - T_rel < delta
                        # (value = -(delta-w0) + t'_rel - T_rel, keep >= 0)
                        delta = Tb * KBLK - tau * TBLK
                        nc.gpsimd.affine_select(
                            out=e[:, w0:], in_=e[:, w0:],
                            compare_op=mybir.AluOpType.is_ge,
                            fill=0.0, base=-(delta - w0),
                            pattern=[[1, TBLK - w0]], channel_multiplier=-1)
                    elif mode == MODE_LOADMASK:
                        nc.vector.tensor_mul(e[:], e[:], mask_tiles[(tau, Tb)])
                    return e, w0

                def emit_od(i, Tb, e, w0):
                    cur, lvl = e, 0
                    while tree and tree[-1][0] == lvl:
                        _, prev = tree.pop()
                        acc = esum_pool.tile([128, TBLK], bf16, tag="esum")
                        nc.vector.tensor_add(acc[:], prev[:], cur[:])
                        cur, lvl = acc, lvl + 1
                    tree.append((lvl, cur))
                    nc.tensor.matmul(
                        od[:, w0:],
                        lhsT=v_sb[:, Tb, h * HD:(h + 1) * HD],
                        rhs=e[:, w0:],
                        start=(i == 0), stop=(i == len(blocks) - 1))

                for i, (Tb, mode) in enumerate(blocks):
                    if i and i % chunk == 0:
                        yield "chunk"
                    e, w0 = emit_score(i, Tb, mode)
                    emit_od(i, Tb, e, w0)
                yield "pre_dn"
                while len(tree) > 1:
                    _, a = tree.pop()
                    _, b2 = tree.pop()
                    acc = esum_pool.tile([128, TBLK], bf16, tag="esum")
                    nc.vector.tensor_add(acc[:], a[:], b2[:])
                    tree.append((99, acc))
                esum = tree.pop()[1]
                dn = ps_score.tile([1, TBLK], fp32, tag="sc")
                nc.tensor.matmul(dn[:], lhsT=ones_bf_sb[:], rhs=esum[:],
                                 start=True, stop=True)
                yield "post_dn"
                if tau == NT - 1 and h == HPC - 1:
                    # final head: leave the output UNNORMALIZED (copied to
                    # SBUF on ACT, ungated by the reciprocal) and instead
                    # transpose the reciprocal into per-partition columns
                    # with tiny K=1 matmuls; the final output blocks fold
                    # the scale in with one fused scalar_tensor_tensor
                    # each, so the 16 tail matmuls never wait on the
                    # reciprocal chain.
                    # o3 copied in 128-col pieces so the first tail
                    # matmul (needs only its own slice) starts ~0.5us
                    # earlier than a monolithic 512-col copy would allow
                    for j in range(TBLK // 128):
                        nc.scalar.activation(
                            out=o3_sb[:, j * 128:(j + 1) * 128],
                            in_=od[:, j * 128:(j + 1) * 128], func=Copy)
                    r = r_pool.tile([1, TBLK], fp32, tag="r")
                    nc.vector.reciprocal_approx_fast(out=r[:], in_=dn[:])
                    rcp = ps_score.tile([128, TBLK], fp32, tag="sc")
                    for j in range(TBLK // 128):
                        nc.tensor.matmul(
                            rcp[:, j:j + 1],
                            lhsT=r[:, j * 128:(j + 1) * 128],
                            rhs=ones_f32_sb[:],
                            start=True, stop=True)
                    nc.vector.tensor_copy(out=rcol_sb[:],
                                          in_=rcp[:, 0:TBLK // 128])
                else:
                    r = r_pool.tile([1, TBLK], fp32, tag="r")
                    nc.vector.reciprocal_approx_fast(out=r[:], in_=dn[:])
                    r_bf = r_pool.tile([1, TBLK], bf16, tag="rb", bufs=1)
                    nc.vector.tensor_copy(out=r_bf[:], in_=r[:])
                    # partition-broadcast of the reciprocal on the (idle)
                    # GpSimd engine -- no DRAM bounce, no PE matmul; bf16
                    # halves the broadcast and the mul's second operand
                    R = R_pool.tile([128, TBLK], bf16, tag="R")
                    nc.gpsimd.partition_broadcast(R[:], r_bf[:])

                    # the normalize mul waits for the broadcast; emit it
                    # from the NEXT head so it doesn't head-of-line block
                    # the DVE queue (esum adds) behind that wait.
                    def _norm(od=od, R=R, h=h, tsl=tsl):
                        nc.vector.tensor_mul(oT_sb[:, h, tsl], od[:], R[:])
                    yield ("norm", _norm)

            Copy = mybir.ActivationFunctionType.Copy

            def emit_out_block(tt, mb, store_eng=None, copy_eng=None):
                # one 128x512 block of the final projection
                ps = ps_work.tile([128, TBLK], fp32, tag="ps")
                for h in range(HPC):
                    nc.tensor.matmul(
                        ps[:],
                        lhsT=oT_sb[:, h, tt * 128:(tt + 1) * 128],
                        rhs=wo_sb[:, h, mb * TBLK:(mb + 1) * TBLK],
                        start=(h == 0), stop=(h == HPC - 1))
                osb = out_pool.tile([128, TBLK], bf16, tag="osb")
                if copy_eng is nc.scalar:
                    nc.scalar.activation(out=osb[:], in_=ps[:], func=Copy)
                else:
                    (copy_eng or nc.vector).tensor_copy(out=osb[:], in_=ps[:])
                (store_eng or nc.sync).dma_start(
                    out.ap()[tt * 128:(tt + 1) * 128,
                             mb * TBLK:(mb + 1) * TBLK],
                    osb[:])

            # ---- emission schedule -----------------------------------
            # Output rows ride as PE filler: rows 0-3 (query block 0) in
            # slot 2, rows 4-11 in slot 3 (tau=3's attention is the most
            # ACT-bound stretch and otherwise starves the PE), rows 12-15
            # after the final head's chain, with stores on the scalar
            # queue so the final reciprocal bounce never queues behind
            # them on sync.
            rows_per_tau = TBLK // 128
            row_slot = {2: [0, 1, 2, 3], 3: [4, 5, 6, 7, 8, 9, 10, 11]}



            # projections for tau=0 run standalone (startup).  q and k
            # are emitted hk-outer with 8 simultaneously-open PSUM groups
            # so each weight tile is consumed for all 8 head-matmuls the
            # moment it lands -- the PE then tracks the two DMA streams
            # (wq on scalar, wk on sync) instead of stalling head-serial.
            qps = [ps_work.tile([128, TBLK], fp32, tag="ps", name=f"qp{i}")
                   for i in range(3)]
            qps.append(ps_acc.tile([128, TBLK], fp32, tag="od", name="qp3"))
            kps = [ps_score.tile([128, TBLK], fp32, tag="sc", name=f"kp{i}")
                   for i in range(3)]
            kps.append(ps_acc.tile([128, TBLK], fp32, tag="od", name="kp3"))
            for hk in range(HKT):
                for h in range(HPC):
                    nc.tensor.matmul(
                        qps[h][:],
                        lhsT=w_chunk("wq", hk)[:, h * HD:(h + 1) * HD],
                        rhs=xt_chunk(0, hk),
                        start=(hk == 0), stop=(hk == HKT - 1))
                for h in range(HPC):
                    nc.tensor.matmul(
                        kps[h][:],
                        lhsT=w_chunk("wk", hk)[:, h * HD:(h + 1) * HD],
                        rhs=xt_chunk(0, hk),
                        start=(hk == 0), stop=(hk == HKT - 1))
            for h in range(HPC):
                if h % 2:
                    nc.scalar.activation(out=qT_sb[:, h, 0:TBLK],
                                         in_=qps[h][:], func=Copy)
                    nc.vector.tensor_copy(out=kT_sb[:, h, 0:TBLK],
                                          in_=kps[h][:])
                else:
                    nc.vector.tensor_copy(out=qT_sb[:, h, 0:TBLK],
                                          in_=qps[h][:])
                    nc.scalar.activation(out=kT_sb[:, h, 0:TBLK],
                                         in_=kps[h][:], func=Copy)
            # tau1's x lands well before tau0's attention fillers start
            # consuming it (the tau-loop emission point is only ~0-6us
            # ahead of first use for tau==0)
            emit_xt_load(1)
            emit_mask_loads(1)
            masks_loaded = {1}

            for tb in range(rows_per_tau):
                emit_v_proj(0, tb)

            emit_mask_loads(0)

            deferred_norm = None
            for tau in range(NT):
                fillers = []
                if tau + 1 < NT:
                    emit_xt_load(tau + 1)
                    if tau + 1 not in masks_loaded:
                        emit_mask_loads(tau + 1)
                        masks_loaded.add(tau + 1)
                    fillers += [lambda h=h, t=tau + 1:
                                emit_qk_proj(t, "wq", h)
                                for h in range(HPC)]
                    fillers += [lambda h=h, t=tau + 1:
                                emit_qk_proj(t, "wk", h)
                                for h in range(HPC)]
                    fillers += [lambda tb=tb, t=tau + 1:
                                emit_v_proj(t, tb)
                                for tb in range(rows_per_tau)]
                for tt in row_slot.get(tau, []):
                    fillers += [lambda tt=tt, mb=mb: emit_out_block(tt, mb)
                                for mb in range(H // TBLK)]
                fill_iter = iter(fillers)

                def fill(n=1):
                    for _ in range(n):
                        f = next(fill_iter, None)
                        if f is None:
                            return
                        f()

                for h in range(HPC):
                    for ev in emit_attention_head(tau, h):
                        if ev in ("chunk", "pre_dn"):
                            if deferred_norm is not None:
                                deferred_norm()
                                deferred_norm = None
                            fill(1)
                        elif isinstance(ev, tuple) and ev[0] == "norm":
                            deferred_norm = ev[1]
                    fill(1)
                for f in fill_iter:
                    f()

            # Final query block's output rows, fully interleaved: per
            # (tt, mb) block, the heads-0-2 partial (3 matmuls + cast
            # into a coalesced row tile, stored to `out`) is followed
            # immediately by the head-3 term against the UNNORMALIZED
            # o3_sb (1 matmul; the per-query reciprocal is folded into
            # the PSUM->SBUF copy and ships via out2).  Interleaving
            # keeps both copy engines draining tail copies from the
            # start instead of queueing them behind all 16 partial
            # casts, so the PSUM rings never pace the matmuls.  The two
            # per-block copies go to opposite engines.
            tail_pools = [(ps_acc, "od"), (ps_score, "sc")]
            j = 0
            for tt in range((NT - 1) * rows_per_tau, NT * rows_per_tau):
                ttl = tt - (NT - 1) * rows_per_tau
                orow = fin_pool.tile([128, H], bf16, tag="fin", bufs=4,
                                     name="orow")
                for mb in range(H // TBLK):
                    ps = ps_work.tile([128, TBLK], fp32, tag="ps")
                    for h in range(HPC - 1):
                        nc.tensor.matmul(
                            ps[:],
                            lhsT=oT_sb[:, h, tt * 128:(tt + 1) * 128],
                            rhs=wo_sb[:, h, mb * TBLK:(mb + 1) * TBLK],
                            start=(h == 0), stop=(h == HPC - 2))
                    osb = orow[:, mb * TBLK:(mb + 1) * TBLK]
                    if j % 2:
                        nc.scalar.activation(out=osb, in_=ps[:], func=Copy)
                    else:
                        nc.vector.tensor_copy(out=osb, in_=ps[:])

                    pool_, tag_ = tail_pools[j % 2]
                    ps2 = pool_.tile([128, TBLK], fp32, tag=tag_, name="ps2")
                    nc.tensor.matmul(
                        ps2[:],
                        lhsT=o3_sb[:, ttl * 128:(ttl + 1) * 128],
                        rhs=wo_sb[:, HPC - 1, mb * TBLK:(mb + 1) * TBLK],
                        start=True, stop=True)
                    if j % 2:
                        o2sb = fin_pool.tile([128, TBLK], bf16, tag="fin2v",
                                             bufs=2)
                        nc.vector.tensor_scalar_mul(
                            out=o2sb[:], in0=ps2[:],
                            scalar1=rcol_sb[:, ttl:ttl + 1])
                    else:
                        o2sb = fin_pool.tile([128, TBLK], bf16, tag="fin2s",
                                             bufs=2)
                        nc.scalar.activation(out=o2sb[:], in_=ps2[:],
                                             func=Copy,
                                             scale=rcol_sb[:, ttl:ttl + 1])
                    (nc.scalar if j % 2 else nc.sync).dma_start(
                        out2.ap()[ttl * 128:(ttl + 1) * 128,
                                  mb * TBLK:(mb + 1) * TBLK],
                        o2sb[:])
                    j += 1
                nc.sync.dma_start(
                    out.ap()[tt * 128:(tt + 1) * 128, :], orow[:])

    nc.compile()
    return nc


def _classify(mask):
    """Per 128x512 block of mask^T: skip / full / affine / partial,
    unioned over batches.  Returns the pattern tuple, or None if some
    row is fully masked (degenerate -- reference gives uniform weights
    there)."""
    if not mask.any(axis=2).all():
        return None
    tr = np.arange(TBLK)[:, None]
    Tr = np.arange(KBLK)[None, :]
    pattern = []
    for tau in range(NT):
        blocks = []
        for Tb in range(NK):
            # block of mask^T[Tb*128:(Tb+1)*128, tau*512:(tau+1)*512]
            # == mask[:, tau*512:(tau+1)*512, Tb*128:(Tb+1)*128]
            blk = mask[:, tau * TBLK:(tau + 1) * TBLK,
                       Tb * KBLK:(Tb + 1) * KBLK]
            if not blk.any():
                continue
            if blk.all():
                blocks.append((Tb, MODE_FULL))
                continue
            # causal staircase? mask[t, T] = (t >= T), i.e.
            # tau*TBLK + tr >= Tb*KBLK + Tr
            stair = (tau * TBLK + tr) >= (Tb * KBLK + Tr)
            if all((blk[b] == stair).all() for b in range(blk.shape[0])):
                blocks.append((Tb, MODE_AFFINE))
            else:
                blocks.append((Tb, MODE_LOADMASK))
        pattern.append(tuple(blocks))
    return tuple(pattern)


def _reference_fallback(x, mask, Wq, Wk, Wv, Wo):
    out = np.empty((B, S, H), np.float32)
    for b in range(B):
        q = (x[b] @ Wq).reshape(S, NH, HD).transpose(1, 0, 2)
        k = (x[b] @ Wk).reshape(S, NH, HD).transpose(1, 0, 2)
        v = (x[b] @ Wv).reshape(S, NH, HD).transpose(1, 0, 2)
        s = np.einsum("htd,hTd->htT", q, k) / np.sqrt(HD)
        s = np.where(mask[b][None], s, -1e10)
        s -= s.max(-1, keepdims=True)
        w = np.exp(s)
        w /= w.sum(-1, keepdims=True)
        o = np.einsum("htT,hTd->htd", w, v)
        out[b] = o.transpose(1, 0, 2).reshape(S, NH * HD) @ Wo
    return out


def kernel(x, mask, Wq, Wk, Wv, Wo):
    x = np.asarray(x, np.float32)
    mask = np.asarray(mask).astype(bool)
    Wq = np.asarray(Wq, np.float32)
    Wk = np.asarray(Wk, np.float32)
    Wv = np.asarray(Wv, np.float32)
    Wo = np.asarray(Wo, np.float32)
    assert x.shape == (B, S, H) and mask.shape == (B, S, S)

    pattern = _classify(mask)
    if pattern is None:
        return _reference_fallback(x, mask, Wq, Wk, Wv, Wo)

    if pattern not in _kernel_cache:
        try:
            _kernel_cache[pattern] = _build(pattern)
        except Exception:
            # exotic mask patterns can exceed the SBUF budget (mask-tile
            # pool); stay correct via the host path
            _kernel_cache[pattern] = None
    nc = _kernel_cache[pattern]
    if nc is None:
        return _reference_fallback(x, mask, Wq, Wk, Wv, Wo)

    xT_b = [np.ascontiguousarray(x[b].T).astype(_BF16) for b in range(B)]
    maskT_b = [np.ascontiguousarray(mask[b].T).astype(_BF16) for b in range(B)]
    wq_g = [np.ascontiguousarray(Wq[:, g * DPC:(g + 1) * DPC]).astype(_BF16)
            for g in range(GROUPS)]
    wk_g = [np.ascontiguousarray(Wk[:, g * DPC:(g + 1) * DPC]).astype(_BF16)
            for g in range(GROUPS)]
    wv_g = [np.ascontiguousarray(Wv[:, g * DPC:(g + 1) * DPC]).astype(_BF16)
            for g in range(GROUPS)]
    wo_g = [np.ascontiguousarray(Wo[g * DPC:(g + 1) * DPC, :]).astype(_BF16)
            for g in range(GROUPS)]

    in_maps = []
    for i in range(N_CORES):
        b, g = divmod(i, GROUPS)
        in_maps.append({
            "xT": xT_b[b], "maskT": maskT_b[b],
            "wq": wq_g[g], "wk": wk_g[g], "wv": wv_g[g], "wo": wo_g[g],
        })

    from concourse.bass_utils import run_bass_kernel_spmd
    res = run_bass_kernel_spmd(nc, in_maps, core_ids=list(range(N_CORES)))

    out = np.zeros((B, S, H), np.float32)
    for i in range(N_CORES):
        b = i // GROUPS
        out[b] += res.results[i]["out"].astype(np.float32)
        # final query block's head-3 term ships separately
        out[b, S - TBLK:] += res.results[i]["out2"].astype(np.float32)
    return out

